# revision 10
# baseline (speedup 1.0000x reference)
"""GRU cell kernel for Trainium2, data-parallel over 8 NeuronCores.

Math (per batch row):
    x_proj = x @ W_ih.T + b           -> r_x, z_x, n_x
    r = sigmoid(r_x + h @ U_r.T)
    z = sigmoid(z_x + h @ U_z.T)
    n = tanh(n_x + r * (h @ U_n.T + U_n_b))
    out = (1 - z) * n + z * h

Layout strategy: all on-chip compute happens in "transposed" orientation so
both matmul operands carry the contraction dim H on the partition axis:
  - host sends x.T, h.T slices per core ([H, B_local]) and pre-packed
    transposed weights; kernel computes out.T tiles [o_feat=128, batch=512]
  - bf16 matmuls (full PE rate), fp32 PSUM accumulation, fp32 epilogue
  - the epilogue reads h from the same bf16 tile the matmuls use (no
    separate fp32 h load) and stores bf16 output; host upcasts/transposes
  - every DMA is contiguous per partition on BOTH the DRAM and SBUF side
    (host packs blobs in exactly the SBUF layout), so transfers run at
    full descriptor size; chunks are ordered/sized by first consumption
"""

import os
import sys
import types

import numpy as np
import ml_dtypes

import concourse.bass as bass
import concourse.mybir as mybir
import concourse.tile as tile
from concourse import bacc
from concourse.bass_utils import run_bass_kernel_spmd


def _ensure_ntff_hook():
    """On images whose ``antenv`` predates ``antenv.axon_hooks``, the traced
    path of ``run_bass_kernel_spmd`` crashes on import (even when tracing is
    merely enabled via the BASS_TRACE env var). Synthesize the module with
    the same ctypes hook the boot code would have registered."""
    try:
        import antenv.axon_hooks  # noqa: F401
        return
    except ImportError:
        pass
    hook = None
    try:
        from trn_agent_boot.trn_boot import _ntff_profile_via_ctypes

        so_path = "/opt/axon/libaxon_pjrt.so"
        if os.path.exists(so_path):
            hook = _ntff_profile_via_ctypes(so_path)
    except Exception:
        hook = None
    mod = types.ModuleType("antenv.axon_hooks")
    mod.get_axon_ntff_profile_hook = lambda: hook
    mod.set_axon_ntff_profile_hook = lambda h: None
    sys.modules["antenv.axon_hooks"] = mod


_ensure_ntff_hook()

H = 1024
B = 8192
NCORES = 8
BL = B // NCORES          # batch rows per core
KT = H // 128             # contraction k-tiles
OT = H // 128             # output-feature tiles (per gate)
NB = BL // 512            # batch slices of 512
F32 = mybir.dt.float32
BF16 = mybir.dt.bfloat16
BF16_NP = ml_dtypes.bfloat16

# weight tile free-dim layout (flat 6144 cols):
#   section A (cols 0:2048):    [k][g][128] for g in (nx, nh)
#   section B (cols 2048:6144): [k][g][128] for g in (rx, rh, zx, zh)
G_NX, G_NH, G_RX, G_RH, G_ZX, G_ZH = range(6)


def _wcol(g, k):
    if g < 2:
        return k * 256 + g * 128
    return 2048 + k * 512 + (g - 2) * 128


# bias columns per o (unchanged from baseline): 0=b_r 1=b_z 2=b_n1 3=b_n2

N_WARM = 6   # PE clock warm-up matmuls; DMA-paced real matmuls finish the ramp

LAST_RESULT = None  # BassKernelResults of the most recent run (for test harness)


def _gru_tile_kernel(tc, outt, xh, wpa, wpb, bias_ap):
    nc = tc.nc
    sig = mybir.ActivationFunctionType.Sigmoid
    tanh = mybir.ActivationFunctionType.Tanh
    add = mybir.AluOpType.add
    mult = mybir.AluOpType.mult

    from contextlib import ExitStack

    with ExitStack() as ctx:
        singles = ctx.enter_context(tc.tile_pool(name="singles", bufs=1))
        wpool = ctx.enter_context(tc.tile_pool(name="wpool", bufs=2))
        gates = ctx.enter_context(tc.tile_pool(name="gates", bufs=2))
        outp = ctx.enter_context(tc.tile_pool(name="outp", bufs=3))
        psum = ctx.enter_context(tc.tile_pool(name="psum", bufs=2, space="PSUM"))

        # resident activations: x.T and h.T bf16, one 4-D tile indexed as
        # [p, region, k, col] with region 0=x_b0 1=h_b0 2=x_b1 3=h_b1;
        # each region is contiguous per partition so its load DMA runs
        # with 4-8KB descriptors
        xh_t = singles.tile([128, 4, KT, 512], BF16, name="xh", tag="xh")
        bias_t = singles.tile([128, OT * 4], F32, name="bias", tag="bias")
        warm_sb = singles.tile([128, 512], BF16, name="warm_sb", tag="warm_sb")

        # warm the PE clock (HAM) with dummy matmuls on a memset tile so
        # the real matmul stream starts closer to full clock; memset rides
        # the otherwise-idle vector engine
        nc.vector.memset(warm_sb[:], 0.0)
        warm_ps = psum.tile([128, 512], F32, name="warm_ps", tag="r_ps")
        for _ in range(N_WARM):
            nc.tensor.matmul(
                warm_ps[:], warm_sb[:, 0:128], warm_sb[:], start=True, stop=True
            )

        # input loads, in consumption order: x/h region chunks ride the
        # sync HWDGE ring; weights ride the scalar HWDGE ring split per o
        # into the critical A section (n gates) and B section (r/z gates);
        # bias rides gpsimd SWDGE
        nc.gpsimd.dma_start(out=bias_t[:], in_=bias_ap[:])
        nc.sync.dma_start(out=xh_t[:, 0, 0:4, :], in_=xh[:, 0, 0:4])
        nc.sync.dma_start(out=xh_t[:, 0, 4:8, :], in_=xh[:, 0, 4:8])
        nc.sync.dma_start(out=xh_t[:, 1, 0:4, :], in_=xh[:, 1, 0:4])
        nc.sync.dma_start(out=xh_t[:, 1, 4:8, :], in_=xh[:, 1, 4:8])
        nc.sync.dma_start(out=xh_t[:, 2], in_=xh[:, 2])
        nc.sync.dma_start(out=xh_t[:, 3], in_=xh[:, 3])

        for o in range(OT):
            # packed weights for this output-feature tile, flat free dim.
            # o=0 rides scalar in fine chunks (it gates the opening);
            # o=1 rides sync BEHIND the xh chunks so it cannot steal HBM
            # bandwidth from the critical h_b0/x_b1 stream; o>=2 ride
            # scalar, naturally delayed by the weight-pool WAR dependency.
            wt = wpool.tile([128, 6 * KT * 128], BF16, name="wt", tag="wt")
            if o == 0:
                nc.scalar.dma_start(out=wt[:, 0:1024], in_=wpa[o][:, 0:1024])
                nc.scalar.dma_start(out=wt[:, 1024:2048], in_=wpa[o][:, 1024:2048])
                nc.scalar.dma_start(out=wt[:, 2048:4096], in_=wpb[o][:, 0:2048])
                nc.scalar.dma_start(out=wt[:, 4096:6144], in_=wpb[o][:, 2048:4096])
            else:
                eng = nc.sync if o == 1 else nc.scalar
                eng.dma_start(out=wt[:, 0:2048], in_=wpa[o])
                eng.dma_start(out=wt[:, 2048:6144], in_=wpb[o])

            for b in range(NB):
                xreg = 2 * b
                hreg = 2 * b + 1
                last_unit = (o == OT - 1) and (b == NB - 1)
                r_ps = psum.tile([128, 512], F32, name="r_ps", tag="r_ps")
                z_ps = psum.tile([128, 512], F32, name="z_ps", tag="z_ps")
                nx_ps = psum.tile([128, 512], F32, name="nx_ps", tag="nx_ps")
                nh_ps = psum.tile([128, 512], F32, name="nh_ps", tag="nh_ps")

                def mm(ps, g, reg, first, last, cols=slice(0, 512), fill=0):
                    # fill: zero-adding matmuls on the memset warm tile,
                    # issued at the head of the group — they keep the PE
                    # busy (and its clock ramping) while this group's
                    # input chunks are still arriving
                    w = cols.stop - cols.start
                    for f in range(fill):
                        nc.tensor.matmul(
                            ps[:, cols], warm_sb[:, 0:128], warm_sb[:, 0:w],
                            start=(first and f == 0), stop=False,
                        )
                    for k in range(KT):
                        nc.tensor.matmul(
                            ps[:, cols],
                            wt[:, _wcol(g, k) : _wcol(g, k) + 128],
                            xh_t[:, reg, k, cols],
                            start=(first and fill == 0 and k == 0),
                            stop=(last and k == KT - 1),
                        )

                # group order: nx | r=(x,h) | nh | z=(x,h). In the very
                # first unit the three x-side halves run before any h-side
                # work so the PE streams while h_b0 is still arriving, with
                # filler matmuls covering the known arrival gaps.
                if o == 0 and b == 0:
                    mm(nx_ps, G_NX, xreg, True, True, fill=2)
                    mm(r_ps, G_RX, xreg, True, False, fill=5)
                    mm(z_ps, G_ZX, xreg, True, False, fill=2)
                    mm(nh_ps, G_NH, hreg, True, True, fill=2)
                    mm(r_ps, G_RH, hreg, False, True)
                else:
                    mm(nx_ps, G_NX, xreg, True, True)
                    mm(r_ps, G_RX, xreg, True, False)
                    mm(r_ps, G_RH, hreg, False, True)
                    mm(nh_ps, G_NH, hreg, True, True)

                r_sb = gates.tile([128, 512], F32, name="r", tag="r")
                nc.scalar.activation(
                    out=r_sb[:], in_=r_ps[:], func=sig,
                    bias=bias_t[:, o * 4 + 0 : o * 4 + 1],
                )

                # z matmuls stream while the n/tanh chain runs on
                # scalar/vector. The last unit computes z in two separate
                # column-half PSUM tiles so the first half's epilogue and
                # store overlap the second half's matmuls.
                if last_unit:
                    zb_ps = psum.tile([128, 256], F32, name="zb_ps", tag="nx_ps")
                    mm(z_ps, G_ZX, xreg, True, False, slice(0, 256))
                    mm(z_ps, G_ZH, hreg, False, True, slice(0, 256))
                    for gi, g in enumerate((G_ZX, G_ZH)):
                        reg = xreg if g == G_ZX else hreg
                        for k in range(KT):
                            nc.tensor.matmul(
                                zb_ps[:],
                                wt[:, _wcol(g, k) : _wcol(g, k) + 128],
                                xh_t[:, reg, k, 256:512],
                                start=(gi == 0 and k == 0),
                                stop=(gi == 1 and k == KT - 1),
                            )
                elif o == 0 and b == 0:
                    mm(z_ps, G_ZH, hreg, False, True)
                else:
                    mm(z_ps, G_ZX, xreg, True, False)
                    mm(z_ps, G_ZH, hreg, False, True)

                # t = (n_h + b_n2) * r ; s = n_x + t ; n = tanh(s + b_n1)
                # d = h - n    (all run while the z matmuls stream)
                t_sb = gates.tile([128, 512], F32, name="t", tag="t")
                nc.vector.scalar_tensor_tensor(
                    out=t_sb[:], in0=nh_ps[:],
                    scalar=bias_t[:, o * 4 + 3 : o * 4 + 4],
                    in1=r_sb[:], op0=add, op1=mult,
                )
                s_sb = gates.tile([128, 512], F32, name="s", tag="s")
                nc.vector.tensor_add(s_sb[:], nx_ps[:], t_sb[:])
                n_sb = gates.tile([128, 512], F32, name="n", tag="n")
                nc.scalar.activation(
                    out=n_sb[:], in_=s_sb[:], func=tanh,
                    bias=bias_t[:, o * 4 + 2 : o * 4 + 3],
                )
                d_sb = gates.tile([128, 512], F32, name="d", tag="d")
                nc.vector.tensor_sub(d_sb[:], xh_t[:, hreg, o, :], n_sb[:])

                # post-z chain in column chunks so the final chunk's serial
                # latency (and the kernel tail) shrinks:
                # z = sigmoid(z_pre + b_z); out = n + z * d
                # The last unit puts a small chunk first so its store (and
                # the exec-ending DMA receipt) is issued as early as
                # possible, with the two stores on different HWDGE rings.
                z_sb = gates.tile([128, 512], F32, name="z", tag="z")
                p_sb = gates.tile([128, 512], F32, name="p", tag="p")
                o_sb = outp.tile([128, 512], BF16, name="o", tag="o")
                if last_unit:
                    chunks = [
                        (slice(0, 256), z_ps[:, 0:256], nc.scalar),
                        (slice(256, 512), zb_ps[:], nc.sync),
                    ]
                else:
                    st = nc.sync if o >= 4 else nc.gpsimd
                    chunks = [
                        (slice(0, 256), z_ps[:, 0:256], st),
                        (slice(256, 512), z_ps[:, 256:512], st),
                    ]
                for cc, z_src, store_eng in chunks:
                    nc.scalar.activation(
                        out=z_sb[:, cc], in_=z_src, func=sig,
                        bias=bias_t[:, o * 4 + 1 : o * 4 + 2],
                    )
                    nc.vector.tensor_mul(p_sb[:, cc], z_sb[:, cc], d_sb[:, cc])
                    nc.vector.tensor_add(o_sb[:, cc], n_sb[:, cc], p_sb[:, cc])
                    store_eng.dma_start(
                        out=outt[o, :, b * 512 + cc.start : b * 512 + cc.stop],
                        in_=o_sb[:, cc],
                    )


_NC_CACHE = None


def _build_nc():
    global _NC_CACHE
    if _NC_CACHE is not None:
        return _NC_CACHE
    nc = bacc.Bacc(
        "TRN2", target_bir_lowering=False, debug=False, num_devices=NCORES
    )
    xh = nc.dram_tensor("xh", [128, 4, KT, 512], BF16, kind="ExternalInput").ap()
    wpa = nc.dram_tensor("wpa", [OT, 128, 2048], BF16, kind="ExternalInput").ap()
    wpb = nc.dram_tensor("wpb", [OT, 128, 4096], BF16, kind="ExternalInput").ap()
    bias = nc.dram_tensor("bias", [128, OT * 4], F32, kind="ExternalInput").ap()
    outt = nc.dram_tensor("outt", [OT, 128, BL], BF16, kind="ExternalOutput").ap()

    with tile.TileContext(nc) as tc:
        _gru_tile_kernel(tc, outt, xh, wpa, wpb, bias)
    nc.compile()
    _NC_CACHE = nc
    return nc


def _pack_inputs(x, h, W_ih_w, W_ih_b, U_r_w, U_z_w, U_n_w, U_n_b):
    x = np.asarray(x, dtype=np.float32)
    h = np.asarray(h, dtype=np.float32)
    xTb = np.ascontiguousarray(x.T).astype(BF16_NP)   # [H, B]
    hTb = np.ascontiguousarray(h.T).astype(BF16_NP)

    W_all = np.concatenate(
        [np.asarray(W_ih_w, np.float32)] +
        [np.asarray(u, np.float32) for u in (U_r_w, U_z_w, U_n_w)],
        axis=0,
    )                                                   # [6H, H] rows: Wr Wz Wn Ur Uz Un
    WT = np.ascontiguousarray(W_all.T)                  # [H, 6H], col blocks same order
    # gate blocks re-ordered to [Wn Un Wr Ur Wz Uz]; per o, section A holds
    # the n gates [k][g][128] and section B the r/z gates, both contiguous
    # per partition
    perm = [2, 5, 0, 3, 1, 4]
    W6 = WT.reshape(H, 6, OT, 128)[:, perm]             # [hrow, g, o, m]
    W6 = W6.reshape(KT, 128, 6, OT, 128).transpose(3, 1, 0, 2, 4)  # [o,p,k,g,m]
    wpa = np.ascontiguousarray(W6[:, :, :, 0:2].reshape(OT, 128, 2048)).astype(BF16_NP)
    wpb = np.ascontiguousarray(W6[:, :, :, 2:6].reshape(OT, 128, 4096)).astype(BF16_NP)

    b_all = np.concatenate(
        [np.asarray(W_ih_b, np.float32), np.asarray(U_n_b, np.float32)]
    )                                                   # [4H]: b_r b_z b_n1 b_n2
    # bias[m, o*4 + g] = b_all[g*H + o*128 + m]
    bias = np.ascontiguousarray(
        b_all.reshape(4, OT, 128).transpose(2, 1, 0).reshape(128, OT * 4)
    ).astype(np.float32)

    in_maps = []
    for c in range(NCORES):
        sl = slice(c * BL, (c + 1) * BL)
        # [p, k, bhalf, col] views of this core's x.T / h.T
        xq = xTb[:, sl].reshape(KT, 128, 2, 512).transpose(1, 0, 2, 3)
        hq = hTb[:, sl].reshape(KT, 128, 2, 512).transpose(1, 0, 2, 3)
        blob = np.empty((128, 4, KT, 512), dtype=BF16_NP)
        blob[:, 0] = xq[:, :, 0]
        blob[:, 1] = hq[:, :, 0]
        blob[:, 2] = xq[:, :, 1]
        blob[:, 3] = hq[:, :, 1]
        in_maps.append({
            "xh": np.ascontiguousarray(blob),
            "wpa": wpa,
            "wpb": wpb,
            "bias": bias,
        })
    return in_maps


def kernel(x, h, W_ih_w, W_ih_b, U_r_w, U_z_w, U_n_w, U_n_b):
    global LAST_RESULT
    nc = _build_nc()
    in_maps = _pack_inputs(x, h, W_ih_w, W_ih_b, U_r_w, U_z_w, U_n_w, U_n_b)
    trace = bool(os.environ.get("GRU_TRACE"))
    res = run_bass_kernel_spmd(nc, in_maps, list(range(NCORES)), trace=trace)
    LAST_RESULT = res
    out = np.empty((B, H), dtype=np.float32)
    for c in range(NCORES):
        ot = res.results[c]["outt"]                     # [OT, 128, BL] bf16
        out[c * BL : (c + 1) * BL, :] = ot.reshape(H, BL).T.astype(np.float32)
    return out
